# revision 2
# baseline (speedup 1.0000x reference)
"""MoE routed conv for Trainium2, 8-core SPMD.

Math: each batch image selects one expert (argmax of scores); output is a
3x3 pad-1 conv of that image with the selected expert's [128,128,3,3] filter.
The dense-conv + mask in the reference is equivalent (forward pass), so we
only compute the selected expert's conv: 5x less work.

Distribution: data-parallel over batch, 4 images per core. Host does the
(trivial) argmax routing and weight gather; the device program is uniform.

Device kernel (per core): shifted-window implicit GEMM.
  - x image lives in SBUF as [128ci, 56h, 58w] (columns padded with zeros).
  - For each of 9 taps (kh,kw): one fp32r matmul per 8-row output chunk,
    accumulating into PSUM: psum[co, h, w] += w_tap[ci,co].T @ x[ci, h+kh-1, w+kw-1].
  - Row clipping at image top/bottom handled by shrinking the matmul row range
    (keeps the PSUM destination full-width: fp32r requires 8B-aligned dst runs).
  - fp32r (TF32-like, 1 col/cycle, 4x faster than fp32) via SWDGE DMA-cast on load.
"""
import numpy as np

B, C, H, W = 32, 128, 56, 56
E, OC = 5, 128
NCORES = 8
IPC = B // NCORES          # images per core
CH = 8                     # output rows per chunk
NCHUNK = H // CH           # 7
WP = W + 2                 # padded width

_program = None


def _build_program():
    import concourse.bacc as bacc
    import concourse.tile as tile
    from concourse import mybir

    dt = mybir.dt
    nc = bacc.Bacc("TRN2", target_bir_lowering=False, debug=False)
    x_d = nc.dram_tensor("x", [IPC, C, H, W], dt.float32, kind="ExternalInput").ap()
    w_d = nc.dram_tensor("w", [IPC, C, 9, OC], dt.float32, kind="ExternalInput").ap()
    z_d = nc.dram_tensor("z", [C, H, 2], dt.float32, kind="ExternalInput").ap()
    o_d = nc.dram_tensor("o", [IPC, OC, H, W], dt.float32, kind="ExternalOutput").ap()

    NXT = 3  # x-tile ring: load-ahead depth

    with tile.TileContext(nc) as tc:
        with (
            tc.tile_pool(name="xp", bufs=1) as xp,
            tc.tile_pool(name="wp", bufs=1) as wpool,
            tc.tile_pool(name="op", bufs=1) as opool,
            tc.tile_pool(name="ps", bufs=8, space="PSUM") as psp,
        ):
            xts = [xp.tile([C, H, WP], dt.float32r, name=f"xt{i}") for i in range(NXT)]
            wts = [wpool.tile([C, 9, OC], dt.float32r, name=f"wt{i}") for i in range(IPC)]
            ots = [opool.tile([OC, H, W], dt.float32, name=f"ot{i}") for i in range(2)]

            # zero the padding columns once per x tile slot
            for t in xts:
                nc.gpsimd.dma_start(out=t[:, :, 0:1], in_=z_d[:, :, 0:1])
                nc.gpsimd.dma_start(out=t[:, :, WP - 1 : WP], in_=z_d[:, :, 1:2])
            # all weights up front (they're small)
            for i in range(IPC):
                nc.gpsimd.dma_start(out=wts[i][:], in_=w_d[i])

            for img in range(IPC):
                xt = xts[img % NXT]
                nc.gpsimd.dma_start(out=xt[:, :, 1 : W + 1], in_=x_d[img])
                wt = wts[img]
                ot = ots[img % 2]

                pss = [psp.tile([OC, CH, W], dt.float32, name=f"ps{img}_{c}", tag="ps")
                       for c in range(NCHUNK)]
                ntap = 9
                for i, (kh, kw) in enumerate(
                    (kh, kw) for kh in range(3) for kw in range(3)
                ):
                    lhsT = wt[:, kh * 3 + kw, :]
                    for c in range(NCHUNK):
                        r0 = c * CH
                        hs = max(r0, 1 - kh)
                        he = min(r0 + CH, H + 1 - kh)
                        rhs = xt[:, hs + kh - 1 : he + kh - 1, kw : kw + W]
                        out = pss[c][:, hs - r0 : he - r0, :]
                        nc.tensor.matmul(out, lhsT, rhs,
                                         start=(i == 0), stop=(i == ntap - 1))
                for c in range(NCHUNK):
                    nc.vector.tensor_copy(ot[:, c * CH : (c + 1) * CH, :], pss[c][:])
                nc.sync.dma_start(out=o_d[img], in_=ot[:])
    nc.compile()
    return nc


def _get_program():
    global _program
    if _program is None:
        _program = _build_program()
    return _program


def kernel(x: np.ndarray, scores: np.ndarray, weight: np.ndarray,
           **run_kwargs) -> np.ndarray:
    from concourse.bass_utils import run_bass_kernel_spmd

    x = np.ascontiguousarray(x, dtype=np.float32)
    scores = np.asarray(scores, dtype=np.float32)
    weight = np.ascontiguousarray(weight, dtype=np.float32)

    expert = np.argmax(scores, axis=1)                       # [B]
    w_all = weight.reshape(E, OC, C, 3, 3)
    w_sel = w_all[expert]                                    # [B, co, ci, kh, kw]
    # lhsT layout: [ci, tap, co]
    w_lhsT = np.ascontiguousarray(
        w_sel.transpose(0, 2, 3, 4, 1).reshape(B, C, 9, OC), dtype=np.float32
    )
    z = np.zeros((C, H, 2), np.float32)

    nc = _get_program()
    in_maps = [
        {"x": x[k * IPC : (k + 1) * IPC], "w": w_lhsT[k * IPC : (k + 1) * IPC], "z": z}
        for k in range(NCORES)
    ]
    res = run_bass_kernel_spmd(nc, in_maps, list(range(NCORES)), **run_kwargs)
    out = np.concatenate([res.results[k]["o"] for k in range(NCORES)], axis=0)
    if run_kwargs:
        kernel.last_results = res
    return out.astype(np.float32)


# revision 3
# speedup vs baseline: 1.4184x; 1.4184x over previous
"""MoE routed conv for Trainium2, 8-core SPMD.

Math: each batch image selects one expert (argmax of scores); the forward
output equals a 3x3 pad-1 conv of that image with the selected expert's
[128,128,3,3] filter (the dense conv + one-hot mask in the reference).
So we compute only the selected expert's conv: 5x less work.

Distribution: data-parallel over batch, 4 images per core. Host does the
(trivial) argmax routing + weight gather; the device program is uniform SPMD.

Device kernel (per core): shifted-window implicit GEMM in fp32r (TF32-like,
full-rate: 1 column/cycle vs 4 for fp32).
  - Host ships x pre-padded to [128ci, 56h, 58w] (zero border columns) and
    pre-rounded to fp32r, so loads are single contiguous HWDGE DMAs.
  - For each of 9 taps (kh,kw), one matmul per 8-row output chunk accumulates
    into PSUM: psum[co, h, w] += w_tap[ci,co].T @ xpad[ci, h+kh-1, w+kw].
  - Row clipping at image top/bottom shrinks the matmul row range; the PSUM
    destination stays full-width (fp32r dst runs must be 8B-aligned).
"""
import numpy as np

B, C, H, W = 32, 128, 56, 56
E, OC = 5, 128
NCORES = 8
IPC = B // NCORES          # images per core
CH = 8                     # output rows per chunk
NCHUNK = H // CH           # 7
WP = W + 2                 # padded width

_program = None


def _round_f32r(a: np.ndarray) -> np.ndarray:
    """Round fp32 -> fp32r storage format (TF32-like, RTNE on low 13 bits).

    The HW requires fp32r matmul operands to be pre-rounded by a producer;
    rounding on host keeps the device loads as plain contiguous copies.
    """
    u = a.astype(np.float32).view(np.uint32)
    u = (u + 0x1000 + ((u >> 13) & 1)) & np.uint32(0xFFFFE000)
    return u.view(np.float32)


def _build_program():
    import concourse.bacc as bacc
    import concourse.tile as tile
    from concourse import mybir

    dt = mybir.dt
    nc = bacc.Bacc("TRN2", target_bir_lowering=False, debug=False)
    x_d = nc.dram_tensor("x", [IPC, C, H, WP], dt.float32r, kind="ExternalInput").ap()
    w_d = nc.dram_tensor("w", [IPC, C, 9, OC], dt.float32r, kind="ExternalInput").ap()
    o_d = nc.dram_tensor("o", [IPC, OC, H, W], dt.float32, kind="ExternalOutput").ap()

    NXT = 3  # x-tile ring depth

    with tile.TileContext(nc) as tc:
        with (
            tc.tile_pool(name="xp", bufs=1) as xp,
            tc.tile_pool(name="wpool", bufs=1) as wpool,
            tc.tile_pool(name="opool", bufs=1) as opool,
            tc.tile_pool(name="ps", bufs=8, space="PSUM") as psp,
        ):
            xts = [xp.tile([C, H, WP], dt.float32r, name=f"xt{i}") for i in range(NXT)]
            wts = [wpool.tile([C, 9, OC], dt.float32r, name=f"wt{i}") for i in range(IPC)]
            ots = [opool.tile([OC, H, W], dt.float32, name=f"ot{i}") for i in range(2)]

            for i in range(IPC):
                nc.sync.dma_start(out=wts[i][:], in_=w_d[i])

            for img in range(IPC):
                xt = xts[img % NXT]
                nc.sync.dma_start(out=xt[:], in_=x_d[img])
                wt = wts[img]
                ot = ots[img % 2]

                pss = [psp.tile([OC, CH, W], dt.float32, name=f"ps{img}_{c}", tag="ps")
                       for c in range(NCHUNK)]
                for i, (kh, kw) in enumerate(
                    (kh, kw) for kh in range(3) for kw in range(3)
                ):
                    lhsT = wt[:, kh * 3 + kw, :]
                    for c in range(NCHUNK):
                        r0 = c * CH
                        hs = max(r0, 1 - kh)
                        he = min(r0 + CH, H + 1 - kh)
                        rhs = xt[:, hs + kh - 1 : he + kh - 1, kw : kw + W]
                        out = pss[c][:, hs - r0 : he - r0, :]
                        nc.tensor.matmul(out, lhsT, rhs, start=(i == 0), stop=(i == 8))
                for c in range(NCHUNK):
                    nc.vector.tensor_copy(ot[:, c * CH : (c + 1) * CH, :], pss[c][:])
                nc.sync.dma_start(out=o_d[img], in_=ot[:])
    nc.compile()
    return nc


def _get_program():
    global _program
    if _program is None:
        _program = _build_program()
    return _program


def kernel(x: np.ndarray, scores: np.ndarray, weight: np.ndarray,
           **run_kwargs) -> np.ndarray:
    from concourse.bass_utils import run_bass_kernel_spmd

    x = np.asarray(x, dtype=np.float32)
    scores = np.asarray(scores, dtype=np.float32)
    weight = np.asarray(weight, dtype=np.float32)

    expert = np.argmax(scores, axis=1)                       # [B]
    w_sel = weight.reshape(E, OC, C, 3, 3)[expert]           # [B, co, ci, kh, kw]
    # lhsT layout: [ci, tap, co]
    w_lhsT = _round_f32r(
        np.ascontiguousarray(w_sel.transpose(0, 2, 3, 4, 1).reshape(B, C, 9, OC))
    )
    xpad = np.zeros((B, C, H, WP), np.float32)
    xpad[:, :, :, 1 : W + 1] = _round_f32r(x)

    nc = _get_program()
    in_maps = [
        {"x": xpad[k * IPC : (k + 1) * IPC], "w": w_lhsT[k * IPC : (k + 1) * IPC]}
        for k in range(NCORES)
    ]
    res = run_bass_kernel_spmd(nc, in_maps, list(range(NCORES)), **run_kwargs)
    out = np.concatenate([res.results[k]["o"] for k in range(NCORES)], axis=0)
    if run_kwargs:
        kernel.last_results = res
    return out.astype(np.float32)


# revision 4
# speedup vs baseline: 1.7017x; 1.1998x over previous
"""MoE routed conv for Trainium2, 8-core SPMD.

Math: each batch image selects one expert (argmax of scores); the forward
output equals a 3x3 pad-1 conv of that image with the selected expert's
[128,128,3,3] filter (the dense conv + one-hot mask in the reference).
So we compute only the selected expert's conv: 5x less work.

Distribution: data-parallel over batch, 4 images per core. Host does the
(trivial) argmax routing + weight gather; the device program is uniform SPMD.

Device kernel (per core): shifted-window implicit GEMM in fp32r (TF32-like,
full-rate: 1 column/cycle vs 4 for fp32).
  - Host ships x pre-padded to [128ci, 56h, 58w] (zero border columns) and
    pre-rounded to fp32r, so loads are single contiguous HWDGE DMAs.
  - For each of 9 taps (kh,kw), one matmul per 8-row output chunk accumulates
    into PSUM: psum[co, h, w] += w_tap[ci,co].T @ xpad[ci, h+kh-1, w+kw].
  - Row clipping at image top/bottom shrinks the matmul row range; the PSUM
    destination stays full-width (fp32r dst runs must be 8B-aligned).
"""
import numpy as np

B, C, H, W = 32, 128, 56, 56
E, OC = 5, 128
NCORES = 8
IPC = B // NCORES          # images per core
CH = 8                     # output rows per chunk
NCHUNK = H // CH           # 7
WP = W + 2                 # padded width

_program = None


def _round_f32r(a: np.ndarray) -> np.ndarray:
    """Round fp32 -> fp32r storage format (TF32-like, RTNE on low 13 bits).

    The HW requires fp32r matmul operands to be pre-rounded by a producer;
    rounding on host keeps the device loads as plain contiguous copies.
    """
    u = a.astype(np.float32).view(np.uint32)
    u = (u + 0x1000 + ((u >> 13) & 1)) & np.uint32(0xFFFFE000)
    return u.view(np.float32)


def _build_program():
    import concourse.bacc as bacc
    import concourse.tile as tile
    from concourse import mybir

    dt = mybir.dt
    nc = bacc.Bacc("TRN2", target_bir_lowering=False, debug=False)
    x_d = nc.dram_tensor("x", [IPC, C, H, WP], dt.float32r, kind="ExternalInput").ap()
    w_d = nc.dram_tensor("w", [IPC, C, 9, OC], dt.float32r, kind="ExternalInput").ap()
    o_d = nc.dram_tensor("o", [IPC, OC, H, W], dt.float32, kind="ExternalOutput").ap()

    NXT = 3  # x-tile ring depth
    XSEGS = [(0, 17), (17, 33), (33, 49), (49, 56)]  # x load row segments

    with tile.TileContext(nc) as tc:
        with (
            tc.tile_pool(name="xp", bufs=1) as xp,
            tc.tile_pool(name="wpool", bufs=1) as wpool,
            tc.tile_pool(name="opool", bufs=1) as opool,
            tc.tile_pool(name="ps", bufs=8, space="PSUM") as psp,
        ):
            xts = [xp.tile([C, H, WP], dt.float32r, name=f"xt{i}") for i in range(NXT)]
            wts = [wpool.tile([C, 9, OC], dt.float32r, name=f"wt{i}") for i in range(IPC)]
            ots = [opool.tile([OC, H, W], dt.float32, name=f"ot{i}") for i in range(2)]

            for img in range(IPC):
                xt = xts[img % NXT]
                wt = wts[img]
                ot = ots[img % 2]
                nc.sync.dma_start(out=wt[:], in_=w_d[img])
                for (ra, rb) in XSEGS:
                    nc.sync.dma_start(out=xt[:, ra:rb, :], in_=x_d[img, :, ra:rb, :])

                for c in range(NCHUNK):
                    r0 = c * CH
                    ps = psp.tile([OC, CH, W], dt.float32, name=f"ps{img}_{c}", tag="ps")
                    for i, (kh, kw) in enumerate(
                        (kh, kw) for kh in range(3) for kw in range(3)
                    ):
                        hs = max(r0, 1 - kh)
                        he = min(r0 + CH, H + 1 - kh)
                        rhs = xt[:, hs + kh - 1 : he + kh - 1, kw : kw + W]
                        out = ps[:, hs - r0 : he - r0, :]
                        nc.tensor.matmul(out, wt[:, kh * 3 + kw, :], rhs,
                                         start=(i == 0), stop=(i == 8))
                    nc.vector.tensor_copy(ot[:, r0 : r0 + CH, :], ps[:])
                    if img < IPC - 1:
                        if c == 3:
                            nc.sync.dma_start(out=o_d[img, :, 0:32, :], in_=ot[:, 0:32, :])
                        elif c == 6:
                            nc.sync.dma_start(out=o_d[img, :, 32:56, :], in_=ot[:, 32:56, :])
                    else:
                        # last image: flush per chunk to shorten the tail
                        nc.sync.dma_start(out=o_d[img, :, r0 : r0 + CH, :],
                                          in_=ot[:, r0 : r0 + CH, :])
    nc.compile()
    return nc


def _get_program():
    global _program
    if _program is None:
        _program = _build_program()
    return _program


def kernel(x: np.ndarray, scores: np.ndarray, weight: np.ndarray,
           **run_kwargs) -> np.ndarray:
    from concourse.bass_utils import run_bass_kernel_spmd

    x = np.asarray(x, dtype=np.float32)
    scores = np.asarray(scores, dtype=np.float32)
    weight = np.asarray(weight, dtype=np.float32)

    expert = np.argmax(scores, axis=1)                       # [B]
    w_sel = weight.reshape(E, OC, C, 3, 3)[expert]           # [B, co, ci, kh, kw]
    # lhsT layout: [ci, tap, co]
    w_lhsT = _round_f32r(
        np.ascontiguousarray(w_sel.transpose(0, 2, 3, 4, 1).reshape(B, C, 9, OC))
    )
    xpad = np.zeros((B, C, H, WP), np.float32)
    xpad[:, :, :, 1 : W + 1] = _round_f32r(x)

    nc = _get_program()
    in_maps = [
        {"x": xpad[k * IPC : (k + 1) * IPC], "w": w_lhsT[k * IPC : (k + 1) * IPC]}
        for k in range(NCORES)
    ]
    res = run_bass_kernel_spmd(nc, in_maps, list(range(NCORES)), **run_kwargs)
    out = np.concatenate([res.results[k]["o"] for k in range(NCORES)], axis=0)
    if run_kwargs:
        kernel.last_results = res
    return out.astype(np.float32)
